# revision 7
# baseline (speedup 1.0000x reference)
"""AxialCrossMamba Trainium2 kernel.

Sharding: 8 cores = 4 directions x 2 batch-halves. Each core runs one
direction's Mamba block (its own weights) over two batches. Host does the
direction permutations (row/col/diag/anti, c-major [C, L] token layouts),
and the final 4-direction sigmoid gate.

Selective scan runs on a hand-built custom DVE op (DUAL_SCAN): the stock
TensorTensorScanArith inserts a one-cycle bubble per element (feedback
spans two ALU stages), so it runs at ~2.16 ns/elem. Removing the bubble
yields z[k] = a[k]*z[k-2] + b[k] at ~1.1 ns/elem — two independent
recurrences over interleaved even/odd elements. States are processed in
pairs (2p, 2p+1) with (t, e) element interleave; B/C tensors are stored
pre-interleaved in DRAM (strided DMA write from the x-proj PSUM extract),
so the broadcast loads stay contiguous. Chunk carries ride in per-pair
boundary columns (a=0, b=carry).
"""

import sys

for _p in ("/opt/trn_rl_repo", "/root/.axon_site/_ro/trn_rl_repo"):
    if _p not in sys.path:
        sys.path.insert(0, _p)

from contextlib import ExitStack

import numpy as np
import ml_dtypes

import concourse.bass as bass
from concourse import bacc
import concourse.mybir as mybir
import concourse.tile as tile
from concourse.bass_utils import run_bass_kernel_spmd

BF16 = ml_dtypes.bfloat16

# Problem constants
B_, C_, H_, W_ = 4, 192, 64, 64
L = H_ * W_          # 4096 tokens
DS, DC = 16, 4       # d_state, d_conv
DI = 2 * C_          # 384 d_inner
DTR = (C_ + 15) // 16  # 12 dt_rank
NB = 2               # batches per core
ND = DI // 128       # 3 d-blocks
N_CORES = 8

AF = mybir.ActivationFunctionType
ALU = mybir.AluOpType
FP32 = mybir.dt.float32
BF = mybir.dt.bfloat16

TC = 512             # t-chunk
NPAIR = 4            # state-pairs per scan group
NG = 2               # scan groups (NG * NPAIR * 2 == DS)
PLEN = 2 + 2 * TC    # per-pair stream length (2 boundary cols + (t,e))


def make_dual_scan_op():
    """Register the bubble-free interleaved affine scan as a custom DVE op.

    Program: two seed uOps load defined values into block1's A-flop, then a
    steady uOp issues one element per cycle: block0 MULT(src0, NEXT_ALU_OUT_A)
    [= a[k] * z[k-2]], block1 ADD(prev, delay0=src1) -> A-flop, blocks 2-7
    bypass to the writeback. Continuous issue makes the A-flop feedback span
    exactly two elements: two interleaved independent recurrences.
    """
    from concourse import dve_ops
    from concourse.dve_spec import Spec, Src0, Src1
    from concourse.dve_uop import (
        DveOpSpec, UopConfig, InpSel, AluInp, DelayInp, OutSel, OutPath,
        Trigger, ENABLE, AluOp as UAluOp,
    )

    name = "DUAL_SCAN_ANT"
    for o in dve_ops.OPS:
        if o.name == name:
            return o

    def ref(in0, in1, s0, s1, imm2):
        P = in0.shape[0]
        a = np.asarray(in0, np.float32).reshape(P, -1)
        b = np.asarray(in1, np.float32).reshape(P, -1)
        n = a.shape[1]

        def vec(v):
            if isinstance(v, np.ndarray):
                return np.asarray(v, np.float32).reshape(P)
            return np.full(P, float(v), np.float32)

        z = [vec(s1), vec(s1)]  # HW: both streams read the last seed value
        out = np.empty((P, n), np.float32)
        for k in range(n):
            zz = a[:, k] * z[k % 2] + b[:, k]
            z[k % 2] = zz
            out[:, k] = zz
        return out.reshape(np.asarray(in0).shape)

    spec = Spec(body=Src0 * Src1, reference=ref)
    op = dve_ops.DveOp(name, spec, subdim=False, uops_sha={})
    dve_ops.OPS.append(op)
    dve_ops.CUSTOM_DVE_SPECS[name] = spec
    row = dve_ops._CUSTOM_DVE_ROW_BASE + len(dve_ops.OPS) - 1
    dve_ops._SUB_OPCODE_FOR_NAME[name] = row

    def seed(const_sel, nxt):
        u = UopConfig()
        u.enable_input(const_sel, 0)
        u.repeat_count = 1
        u.trigger = (Trigger.COUNT, Trigger.NONE, Trigger.NONE)
        u.next_uop = (nxt, 0, 0)
        u.datapath_config[0].pass_through_alu()
        u.datapath_config[1].pass_through_alu()
        u.datapath_config[1].alu_out_a_enable = ENABLE
        return u

    steady = UopConfig()
    steady.enable_input(InpSel.SRC_0, 0)
    steady.enable_input(InpSel.SRC_1, 1)
    steady.require_inp0 = ENABLE
    steady.require_inp1 = ENABLE
    steady.trigger = (Trigger.SRC_TENSOR_DONE, Trigger.NONE, Trigger.NONE)
    steady.next_uop = (0, 0, 0)
    steady.enable_output(OutSel.ALU_OUT, OutPath.WR0_LO)
    dp = steady.datapath_config
    dp[0].enable_alu(UAluOp.MULTIPLY, AluInp.PREV_ALU_OUT, AluInp.NEXT_ALU_OUT_A)
    dp[0].enable_delay_from_src(DelayInp.PREV_DELAY, 0)
    dp[1].enable_alu(UAluOp.ADD, AluInp.PREV_ALU_OUT, AluInp.PREV_DELAY_0)
    dp[1].alu_out_a_enable = ENABLE
    for i in range(2, 8):
        dp[i].pass_through_alu()

    uops = [seed(InpSel.CONST_0, 1), seed(InpSel.CONST_1, 2), steady]
    for u in uops:
        u.validate("v3")
    dve_ops._COMPILE_CACHE[(name, "v3")] = DveOpSpec(
        name=name, opcode=row, uops=uops, rd1_en=True)
    return op


def build_nc(L=L, TC=TC):
    """Build the SPMD single-core program (identical on all 8 cores)."""
    scan_op = make_dual_scan_op()
    nc = bacc.Bacc("TRN2", debug=False)

    # ---- DRAM I/O ----
    tokT = nc.dram_tensor("tokT", [NB, C_, L], BF, kind="ExternalInput").ap()
    Win = nc.dram_tensor("Win", [C_, 2 * DI], BF, kind="ExternalInput").ap()
    convd = nc.dram_tensor("convd", [ND, DC, 128, 128], BF, kind="ExternalInput").ap()
    convb = nc.dram_tensor("convb", [DI, 1], FP32, kind="ExternalInput").ap()
    Wx = nc.dram_tensor("Wx", [DI, 96], BF, kind="ExternalInput").ap()
    Wdt = nc.dram_tensor("Wdt", [DTR, DI], BF, kind="ExternalInput").ap()
    bdt = nc.dram_tensor("bdt", [DI, 1], FP32, kind="ExternalInput").ap()
    Acoef = nc.dram_tensor("Acoef", [DI, DS], FP32, kind="ExternalInput").ap()
    Dsk = nc.dram_tensor("Dsk", [DI, 1], FP32, kind="ExternalInput").ap()
    Wout = nc.dram_tensor("Wout", [DI, C_], BF, kind="ExternalInput").ap()
    outT = nc.dram_tensor("outT", [NB, C_, L], FP32, kind="ExternalOutput").ap()
    # scratch
    z_scr = nc.dram_tensor("z_scr", [NB, ND, 128, L], BF, kind="Internal").ap()
    y_scr = nc.dram_tensor("y_scr", [NB, ND, 128, L], BF, kind="Internal").ap()
    # B/C per chunk, stored (g, p, t, e)-interleaved for the scan layout
    bc_scr = nc.dram_tensor("bc_scr", [NB, 2, L // TC, DS * TC], BF,
                            kind="Internal").ap()

    io = dict(tokT=tokT, Win=Win, convd=convd, convb=convb, Wx=Wx, Wdt=Wdt,
              bdt=bdt, Acoef=Acoef, Dsk=Dsk, Wout=Wout, outT=outT,
              z_scr=z_scr, y_scr=y_scr, bc_scr=bc_scr)
    with tile.TileContext(nc) as tc:
        with ExitStack() as ctx:
            _emit(ctx, tc, nc, io, scan_op, L=L, TC=TC)
    nc.compile()
    return nc


def _emit(ctx, tc, nc, io, scan_op, *, L, TC):
    tokT, Win, convd, convb, Wx, Wdt, bdt = (
        io["tokT"], io["Win"], io["convd"], io["convb"], io["Wx"], io["Wdt"],
        io["bdt"])
    Acoef, Dsk, Wout, outT = io["Acoef"], io["Dsk"], io["Wout"], io["outT"]
    z_scr, y_scr, bc_scr = io["z_scr"], io["y_scr"], io["bc_scr"]

    P = 128
    NCH = L // TC          # t-chunks
    NN = max(1, L // 512)  # matmul n-chunks
    NSZ = L // NN

    # ---- pools ----
    wp = ctx.enter_context(tc.tile_pool(name="weights", bufs=1))
    big = ctx.enter_context(tc.tile_pool(name="big", bufs=4))    # bf16 [128,L]
    af = ctx.enter_context(tc.tile_pool(name="af", bufs=2))      # fp32 scan a
    bh = ctx.enter_context(tc.tile_pool(name="bh", bufs=2))      # bf16 b_/h_
    bcp = ctx.enter_context(tc.tile_pool(name="bcp", bufs=3))    # bf16 B/C rep
    post = ctx.enter_context(tc.tile_pool(name="post", bufs=3))  # hcm/t4
    uvp = ctx.enter_context(tc.tile_pool(name="uvp", bufs=3))    # uv dup
    dtp = ctx.enter_context(tc.tile_pool(name="dtp", bufs=1))    # dt bf16 resident
    xsp = ctx.enter_context(tc.tile_pool(name="xsp", bufs=1))    # xs bf16 resident
    sm = ctx.enter_context(tc.tile_pool(name="small", bufs=2))
    smE = ctx.enter_context(tc.tile_pool(name="smallE", bufs=2))
    pp = ctx.enter_context(tc.tile_pool(name="psum", bufs=2, space="PSUM"))
    pp2 = ctx.enter_context(tc.tile_pool(name="psum2", bufs=2, space="PSUM"))

    # ---- load weights ----
    win0 = wp.tile([P, 2 * DI], BF, tag="win0")
    win1 = wp.tile([C_ - P, 2 * DI], BF, tag="win1")
    nc.sync.dma_start(win0[:], Win[0:P, :])
    nc.sync.dma_start(win1[:], Win[P:C_, :])
    wdt_full = wp.tile([DTR, DI], BF, tag="wdt")
    nc.sync.dma_start(wdt_full[:], Wdt[:])
    wxs, cw3, cb3, bdt3, ac3, dsk3, wo3 = [], [], [], [], [], [], []
    for db in range(ND):
        r = slice(db * P, (db + 1) * P)
        w1 = wp.tile([P, 96], BF, tag=f"wx{db}")
        nc.sync.dma_start(w1[:], Wx[r, :]); wxs.append(w1)
        wconv = []
        for k in range(DC):
            wck = wp.tile([P, P], BF, tag=f"cw{db}_{k}", name=f"cw{db}_{k}")
            nc.sync.dma_start(wck[:], convd[db, k])
            wconv.append(wck)
        cw3.append(wconv)
        w3 = wp.tile([P, 1], FP32, tag=f"cb{db}")
        nc.sync.dma_start(w3[:], convb[r, :]); cb3.append(w3)
        w4 = wp.tile([P, 1], FP32, tag=f"bdt{db}")
        nc.sync.dma_start(w4[:], bdt[r, :]); bdt3.append(w4)
        w5 = wp.tile([P, DS], FP32, tag=f"ac{db}")
        nc.sync.dma_start(w5[:], Acoef[r, :]); ac3.append(w5)
        w6 = wp.tile([P, 1], FP32, tag=f"dsk{db}")
        nc.sync.dma_start(w6[:], Dsk[r, :]); dsk3.append(w6)
        w7 = wp.tile([P, C_], BF, tag=f"wo{db}")
        nc.sync.dma_start(w7[:], Wout[r, :]); wo3.append(w7)

    # persistent scan carries per (db, g): [P, NPAIR, 2]
    hcarry = {}
    for db in range(ND):
        for g in range(NG):
            hcarry[(db, g)] = sm.tile([P, NPAIR, 2], BF, tag=f"carry{db}_{g}",
                                      name=f"carry{db}{g}", bufs=1)

    for j in range(NB):
        # ================= A: in-proj (+ conv interleaved) =================
        tok0 = big.tile([P, L], BF, tag="big")
        tok1 = big.tile([C_ - P, L], BF, tag="big")
        nc.sync.dma_start(tok0[:], tokT[j, 0:P, :])
        nc.sync.dma_start(tok1[:], tokT[j, P:C_, :])

        xs = []
        for m in range(2 * DI // P):   # M-blocks of xz^T; 0..2 -> xi, 3..5 -> z
            if m < ND:
                xi = big.tile([P, L + DC], BF, tag="big")
                nc.scalar.memzero(xi[:, 0:DC])
            mm = slice(m * P, (m + 1) * P)
            for n in range(NN):
                ns = slice(n * NSZ, (n + 1) * NSZ)
                ps = pp.tile([P, NSZ], FP32, tag="ps")
                nc.tensor.matmul(ps[:], win0[:, mm], tok0[:, ns],
                                 start=True, stop=False)
                nc.tensor.matmul(ps[:], win1[:, mm], tok1[:, ns],
                                 start=False, stop=True)
                if m < ND:
                    nc.scalar.copy(xi[:, DC + n * NSZ: DC + (n + 1) * NSZ],
                                   ps[:])
                else:
                    zt = smE.tile([P, NSZ], BF, tag="ztmp", bufs=2)
                    nc.scalar.copy(zt[:], ps[:])
                    sgz = smE.tile([P, NSZ], BF, tag="sgza", bufs=1)
                    nc.scalar.activation(sgz[:], zt[:], AF.Sigmoid)
                    nc.gpsimd.tensor_tensor(zt[:], zt[:], sgz[:], ALU.mult)
                    nc.sync.dma_start(z_scr[j, m - ND, :, ns], zt[:])
            if m < ND:
                # conv on PE via diagonal weight matrices, then silu
                db = m
                x_ = xsp.tile([P, L], BF, tag=f"xs{db}")
                for n in range(NN):
                    ns = slice(n * NSZ, (n + 1) * NSZ)
                    psc = pp.tile([P, NSZ], FP32, tag="psc")
                    for k in range(DC):
                        nc.tensor.matmul(
                            psc[:], cw3[db][k][:],
                            xi[:, 1 + k + n * NSZ: 1 + k + n * NSZ + NSZ],
                            start=(k == 0), stop=(k == DC - 1))
                    sgt = smE.tile([P, NSZ], BF, tag="sgt", bufs=1)
                    nc.scalar.activation(sgt[:], psc[:], AF.Sigmoid,
                                         bias=cb3[db])
                    nc.vector.scalar_tensor_tensor(x_[:, ns], psc[:],
                                                   cb3[db][:], sgt[:],
                                                   ALU.add, ALU.mult)
                xs.append(x_)

        # ================= C: dbc, dt =================
        dtf = []
        for db in range(ND):
            d_ = dtp.tile([P, L], BF, tag=f"dt{db}", name=f"dt{db}")
            dtf.append(d_)
        for n in range(NN):
            ns = slice(n * NSZ, (n + 1) * NSZ)
            psd = pp2.tile([96, NSZ], FP32, tag="psd")
            for db in range(ND):
                nc.tensor.matmul(psd[:], wxs[db][:], xs[db][:, ns],
                                 start=(db == 0), stop=(db == ND - 1))
            dtl = sm.tile([DTR, NSZ], BF, tag="dtl", bufs=2)
            nc.scalar.copy(dtl[:], psd[0:DTR, :])
            bt = smE.tile([DS, NSZ], BF, tag="bt", bufs=1)
            ct = smE.tile([DS, NSZ], BF, tag="ct", bufs=1)
            nc.vector.tensor_copy(bt[:], psd[32:32 + DS, :])
            nc.vector.tensor_copy(ct[:], psd[64:64 + DS, :])
            # write (g, p, t, e)-interleaved: flat = g*8T + p*2T + 2t + e,
            # source row s = 8g + 2p + e
            for half, src in ((0, bt), (1, ct)):
                dst = bc_scr[j, half, n].rearrange(
                    "(g p t e) -> g p e t", g=NG, p=NPAIR, t=TC, e=2)
                for g in range(NG):
                    for p_ in range(NPAIR):
                        nc.sync.dma_start(dst[g, p_],
                                          src[8 * g + 2 * p_: 8 * g + 2 * p_ + 2, :])
            for db in range(ND):
                psm = pp.tile([P, NSZ], FP32, tag="ps")
                nc.tensor.matmul(psm[:], wdt_full[:, db * P:(db + 1) * P],
                                 dtl[:], start=True, stop=True)
                ec = smE.tile([P, NSZ], FP32, tag="esp", bufs=1)
                nc.scalar.activation(ec[:], psm[:], AF.Exp, bias=bdt3[db])
                nc.scalar.activation(dtf[db][:, ns], ec[:], AF.Ln, bias=1.0)

        # ================= D: selective scan =================
        for ch in range(NCH):
            cs = slice(ch * TC, (ch + 1) * TC)
            # u = dt*xs duplicated into (t, e) pairs, per db
            uvd = []
            for db in range(ND):
                uv = uvp.tile([P, TC, 2], BF, tag="uv")
                nc.vector.tensor_tensor(
                    uv[:],
                    dtf[db][:, cs].unsqueeze(2).broadcast_to((P, TC, 2)),
                    xs[db][:, cs].unsqueeze(2).broadcast_to((P, TC, 2)),
                    ALU.mult)
                uvd.append(uv)
            ysums = {}
            for g in range(NG):
                brep = bcp.tile([P, NPAIR, 2 * TC], BF, tag="bcrep")
                crep = bcp.tile([P, NPAIR, 2 * TC], BF, tag="bcrep")
                bsl = bc_scr[j, 0, ch].rearrange("(g x) -> g x", g=NG)[g] \
                    .rearrange("(p x) -> p x", p=NPAIR)
                csl = bc_scr[j, 1, ch].rearrange("(g x) -> g x", g=NG)[g] \
                    .rearrange("(p x) -> p x", p=NPAIR)
                nc.sync.dma_start(
                    brep[:], bsl.unsqueeze(0).broadcast_to((P, NPAIR, 2 * TC)))
                nc.sync.dma_start(
                    crep[:], csl.unsqueeze(0).broadcast_to((P, NPAIR, 2 * TC)))
                for db in range(ND):
                    # a coefficients: exp(A[s] * dt), strided (t, e) writes
                    a_ = af.tile([P, NPAIR, PLEN], FP32, tag="a")
                    nc.scalar.memzero(a_[:, :, 0:2])
                    av = a_[:, :, 2:].rearrange("p q (t e) -> p q t e", e=2)
                    din = dtf[db][:, cs].unsqueeze(2)
                    for p_ in range(NPAIR):
                        for e in range(2):
                            s = 8 * g + 2 * p_ + e
                            nc.scalar.activation(av[:, p_, :, e:e + 1], din,
                                                 AF.Exp,
                                                 scale=ac3[db][:, s:s + 1])
                    # b = u*B (+ carry boundary)
                    b_ = bh.tile([P, NPAIR, PLEN], BF, tag="bh")
                    nc.vector.tensor_tensor(
                        b_[:, :, 2:],
                        uvd[db][:].rearrange("p t e -> p (t e)").unsqueeze(1)
                        .broadcast_to((P, NPAIR, 2 * TC)),
                        brep[:], ALU.mult)
                    if ch == 0:
                        nc.vector.memset(b_[:, :, 0:2], 0.0)
                    else:
                        nc.vector.tensor_copy(b_[:, :, 0:2],
                                              hcarry[(db, g)][:])
                    # dual interleaved scan
                    h_ = bh.tile([P, NPAIR, PLEN], BF, tag="bh")
                    nc.vector._custom_dve(
                        scan_op,
                        out=h_[:].rearrange("p q t -> p (q t)"),
                        in0=a_[:].rearrange("p q t -> p (q t)"),
                        in1=b_[:].rearrange("p q t -> p (q t)"),
                        s0=0.0, s1=0.0)
                    nc.vector.tensor_copy(hcarry[(db, g)][:],
                                          h_[:, :, 2 * TC:2 * TC + 2])
                    # y partials: sum_s h*C
                    hcm = post.tile([P, NPAIR, 2 * TC], BF, tag="hcm", bufs=1)
                    nc.vector.tensor_tensor(hcm[:], h_[:, :, 2:], crep[:],
                                            ALU.mult)
                    hv = hcm[:].rearrange("p q (t e) -> p q t e", e=2)
                    t4 = post.tile([P, NPAIR, TC], BF, tag="t4", bufs=1)
                    nc.vector.tensor_tensor(t4[:], hv[:, :, :, 0],
                                            hv[:, :, :, 1], ALU.add)
                    t2 = sm.tile([P, 2, TC], BF, tag="t2", bufs=1)
                    nc.vector.tensor_tensor(t2[:], t4[:, 0:2, :],
                                            t4[:, 2:4, :], ALU.add)
                    yg = sm.tile([P, TC], BF, tag=f"yg{db}_{g}", bufs=1)
                    nc.vector.tensor_tensor(yg[:], t2[:, 0, :], t2[:, 1, :],
                                            ALU.add)
                    ysums[(db, g)] = yg
            for db in range(ND):
                ysum = sm.tile([P, TC], BF, tag="ysum", bufs=1)
                nc.vector.tensor_tensor(ysum[:], ysums[(db, 0)][:],
                                        ysums[(db, 1)][:], ALU.add)
                # y = ys + xs*D -> bf16 -> DRAM
                ybf = sm.tile([P, TC], BF, tag="ybf", bufs=2)
                nc.vector.scalar_tensor_tensor(ybf[:], xs[db][:, cs],
                                               dsk3[db][:], ysum[:],
                                               ALU.mult, ALU.add)
                nc.sync.dma_start(y_scr[j, db, :, cs], ybf[:])

    # ================= E: gate + out-proj =================
    for j in range(NB):
        for n in range(NN):
            ns = slice(n * NSZ, (n + 1) * NSZ)
            ygs = []
            for db in range(ND):
                zt = smE.tile([P, NSZ], BF, tag="ze", bufs=2)
                nc.sync.dma_start(zt[:], z_scr[j, db, :, ns])
                yt = smE.tile([P, NSZ], BF, tag="ye", bufs=2)
                nc.sync.dma_start(yt[:], y_scr[j, db, :, ns])
                nc.gpsimd.tensor_tensor(yt[:], yt[:], zt[:], ALU.mult)
                ygs.append(yt)
            for m in range(2):
                msz = P if m == 0 else C_ - P
                mm = slice(m * P, m * P + msz)
                pso = pp2.tile([msz, NSZ], FP32, tag="pso")
                for db in range(ND):
                    nc.tensor.matmul(pso[:], wo3[db][:, mm], ygs[db][:],
                                     start=(db == 0), stop=(db == ND - 1))
                ot = smE.tile([msz, NSZ], FP32, tag="oe", bufs=1)
                nc.scalar.copy(ot[:], pso[:])
                nc.sync.dma_start(outT[j, mm, ns], ot[:])


# ---------------- host side ----------------

_CACHE = {}
PROFILE = False
PROFILE_KW = {}


def _get_nc():
    if "nc" not in _CACHE:
        _CACHE["nc"] = build_nc()
    return _CACHE["nc"]


def _permute_toks(x, idx):
    """x: [C, H, W] fp32 -> 4 direction token maps, each [C, L] (c-major)."""
    c = x.shape[0]
    row = x.reshape(c, -1)
    col = x.transpose(0, 2, 1).reshape(c, -1)
    diag = row[:, idx]
    anti = x[:, :, ::-1].reshape(c, -1)[:, idx]
    return [row, col, diag, anti]


def _unpermute(outs, inv_idx, h, w):
    """outs: list of 4 [C, L] -> sum of un-permuted direction outputs."""
    c = outs[0].shape[0]
    row_f = outs[0].reshape(c, h, w)
    col_f = outs[1].reshape(c, w, h).transpose(0, 2, 1)
    diag_f = outs[2][:, inv_idx].reshape(c, h, w)
    anti_f = outs[3][:, inv_idx].reshape(c, h, w)[:, :, ::-1]
    return row_f + col_f + diag_f + anti_f


def _pack_convd(cw):
    """Per d-block, per tap: diag(conv_w[:, k]) as bf16 PE weights."""
    out = np.zeros((ND, DC, 128, 128), np.float32)
    for db in range(ND):
        for k in range(DC):
            np.fill_diagonal(out[db, k], cw[db * 128:(db + 1) * 128, k])
    return out.astype(BF16)


def _pack_wx(wx):
    """Pad W_x columns so dt/B/C rows land at PSUM partitions 0/32/64."""
    out = np.zeros((DI, 96), np.float32)
    out[:, 0:DTR] = wx[:, 0:DTR]
    out[:, 32:32 + DS] = wx[:, DTR:DTR + DS]
    out[:, 64:64 + DS] = wx[:, DTR + DS:]
    return out.astype(BF16)


def kernel(x, W_in, conv_w, conv_b, W_x, W_dt, b_dt, A_log, D_skip, W_out,
           idx, inv_idx):
    x = np.asarray(x, np.float32)
    idx = np.asarray(idx, np.int32)
    inv_idx = np.asarray(inv_idx, np.int32)
    A = -np.exp(np.asarray(A_log, np.float32))        # [4, DI, DS]
    conv_b = np.asarray(conv_b, np.float32)
    b_dt = np.asarray(b_dt, np.float32)
    D_skip = np.asarray(D_skip, np.float32)

    nc = _get_nc()
    in_maps = []
    for core in range(N_CORES):
        d = core // 2      # direction
        bh = core % 2      # batch half
        toks = np.empty((NB, C_, L), BF16)
        for jb in range(NB):
            b = bh * NB + jb
            toks[jb] = _permute_toks(x[b], idx)[d].astype(BF16)
        in_maps.append(dict(
            tokT=toks,
            Win=np.asarray(W_in[d], np.float32).astype(BF16),
            convd=_pack_convd(np.asarray(conv_w[d], np.float32)),
            convb=np.ascontiguousarray(conv_b[d].reshape(DI, 1)),
            Wx=_pack_wx(np.asarray(W_x[d], np.float32)),
            Wdt=np.asarray(W_dt[d], np.float32).astype(BF16),
            bdt=np.ascontiguousarray(b_dt[d].reshape(DI, 1)),
            Acoef=np.ascontiguousarray(A[d]),
            Dsk=np.ascontiguousarray(D_skip[d].reshape(DI, 1)),
            Wout=np.asarray(W_out[d], np.float32).astype(BF16),
        ))

    res = run_bass_kernel_spmd(nc, in_maps, list(range(N_CORES)),
                               trace=PROFILE, **PROFILE_KW)
    _CACHE["last_exec_ns"] = res.exec_time_ns
    outs = res.results

    # gather: per batch b, the 4 direction outputs live on cores d*2 + b//2
    acc = np.zeros((B_, C_, H_, W_), np.float32)
    for b in range(B_):
        bh, jb = b // NB, b % NB
        douts = [np.asarray(outs[d * 2 + bh]["outT"][jb], np.float32)
                 for d in range(4)]
        acc[b] = _unpermute(douts, inv_idx, H_, W_)
    gate = 1.0 / (1.0 + np.exp(-0.25 * acc))
    return x * gate


# revision 8
# speedup vs baseline: 4.9034x; 4.9034x over previous
"""AxialCrossMamba Trainium2 kernel.

Sharding: 8 cores = 4 directions x 2 batch-halves. Each core runs one
direction's Mamba block (its own weights) over two batches. Host does the
direction permutations (row/col/diag/anti, c-major [C, L] token layouts),
and the final 4-direction sigmoid gate.

Selective scan runs on a hand-built custom DVE op (DUAL_SCAN): the stock
TensorTensorScanArith inserts a one-cycle bubble per element (feedback
spans two ALU stages), so it runs at ~2.16 ns/elem. Removing the bubble
yields z[k] = a[k]*z[k-2] + b[k] at ~1.1 ns/elem — two independent
recurrences over interleaved even/odd elements. States are processed in
pairs (2p, 2p+1) with (t, e) element interleave; B/C tensors are stored
pre-interleaved in DRAM (strided DMA write from the x-proj PSUM extract),
so the broadcast loads stay contiguous. Chunk carries ride in per-pair
boundary columns (a=0, b=carry).
"""

import sys

for _p in ("/opt/trn_rl_repo", "/root/.axon_site/_ro/trn_rl_repo"):
    if _p not in sys.path:
        sys.path.insert(0, _p)

from contextlib import ExitStack

import numpy as np
import ml_dtypes

import concourse.bass as bass
from concourse import bacc
import concourse.mybir as mybir
import concourse.tile as tile
from concourse.bass_utils import run_bass_kernel_spmd

BF16 = ml_dtypes.bfloat16

# Problem constants
B_, C_, H_, W_ = 4, 192, 64, 64
L = H_ * W_          # 4096 tokens
DS, DC = 16, 4       # d_state, d_conv
DI = 2 * C_          # 384 d_inner
DTR = (C_ + 15) // 16  # 12 dt_rank
NB = 2               # batches per core
ND = DI // 128       # 3 d-blocks
N_CORES = 8

AF = mybir.ActivationFunctionType
ALU = mybir.AluOpType
FP32 = mybir.dt.float32
BF = mybir.dt.bfloat16

TC = 512             # t-chunk
NPAIR = 4            # state-pairs per scan group
NG = 2               # scan groups (NG * NPAIR * 2 == DS)
PLEN = 2 + 2 * TC    # per-pair stream length (2 boundary cols + (t,e))


def make_dual_scan_op():
    """Register the bubble-free interleaved affine scan as a custom DVE op.

    Program: two seed uOps load defined values into block1's A-flop, then a
    steady uOp issues one element per cycle: block0 MULT(src0, NEXT_ALU_OUT_A)
    [= a[k] * z[k-2]], block1 ADD(prev, delay0=src1) -> A-flop, blocks 2-7
    bypass to the writeback. Continuous issue makes the A-flop feedback span
    exactly two elements: two interleaved independent recurrences.
    """
    from concourse import dve_ops
    from concourse.dve_spec import Spec, Src0, Src1
    from concourse.dve_uop import (
        DveOpSpec, UopConfig, InpSel, AluInp, DelayInp, OutSel, OutPath,
        Trigger, ENABLE, AluOp as UAluOp,
    )

    name = "DUAL_SCAN_ANT"
    for o in dve_ops.OPS:
        if o.name == name:
            return o

    def ref(in0, in1, s0, s1, imm2):
        P = in0.shape[0]
        a = np.asarray(in0, np.float32).reshape(P, -1)
        b = np.asarray(in1, np.float32).reshape(P, -1)
        n = a.shape[1]

        def vec(v):
            if isinstance(v, np.ndarray):
                return np.asarray(v, np.float32).reshape(P)
            return np.full(P, float(v), np.float32)

        z = [vec(s1), vec(s1)]  # HW: both streams read the last seed value
        out = np.empty((P, n), np.float32)
        for k in range(n):
            zz = a[:, k] * z[k % 2] + b[:, k]
            z[k % 2] = zz
            out[:, k] = zz
        return out.reshape(np.asarray(in0).shape)

    spec = Spec(body=Src0 * Src1, reference=ref)
    op = dve_ops.DveOp(name, spec, subdim=False, uops_sha={})
    dve_ops.OPS.append(op)
    dve_ops.CUSTOM_DVE_SPECS[name] = spec
    row = dve_ops._CUSTOM_DVE_ROW_BASE + len(dve_ops.OPS) - 1
    dve_ops._SUB_OPCODE_FOR_NAME[name] = row

    def seed(const_sel, nxt):
        u = UopConfig()
        u.enable_input(const_sel, 0)
        u.repeat_count = 1
        u.trigger = (Trigger.COUNT, Trigger.NONE, Trigger.NONE)
        u.next_uop = (nxt, 0, 0)
        u.datapath_config[0].pass_through_alu()
        u.datapath_config[1].pass_through_alu()
        u.datapath_config[1].alu_out_a_enable = ENABLE
        return u

    steady = UopConfig()
    steady.enable_input(InpSel.SRC_0, 0)
    steady.enable_input(InpSel.SRC_1, 1)
    steady.require_inp0 = ENABLE
    steady.require_inp1 = ENABLE
    steady.trigger = (Trigger.SRC_TENSOR_DONE, Trigger.NONE, Trigger.NONE)
    steady.next_uop = (0, 0, 0)
    steady.enable_output(OutSel.ALU_OUT, OutPath.WR0_LO)
    dp = steady.datapath_config
    dp[0].enable_alu(UAluOp.MULTIPLY, AluInp.PREV_ALU_OUT, AluInp.NEXT_ALU_OUT_A)
    dp[0].enable_delay_from_src(DelayInp.PREV_DELAY, 0)
    dp[1].enable_alu(UAluOp.ADD, AluInp.PREV_ALU_OUT, AluInp.PREV_DELAY_0)
    dp[1].alu_out_a_enable = ENABLE
    for i in range(2, 8):
        dp[i].pass_through_alu()

    uops = [seed(InpSel.CONST_0, 1), seed(InpSel.CONST_1, 2), steady]
    for u in uops:
        u.validate("v3")
    dve_ops._COMPILE_CACHE[(name, "v3")] = DveOpSpec(
        name=name, opcode=row, uops=uops, rd1_en=True)
    return op


def build_nc(L=L, TC=TC):
    """Build the SPMD single-core program (identical on all 8 cores)."""
    scan_op = make_dual_scan_op()
    nc = bacc.Bacc("TRN2", debug=False)

    # ---- DRAM I/O ----
    tokT = nc.dram_tensor("tokT", [NB, C_, L], BF, kind="ExternalInput").ap()
    Win = nc.dram_tensor("Win", [C_, 2 * DI], BF, kind="ExternalInput").ap()
    convd = nc.dram_tensor("convd", [ND, DC, 128, 128], BF, kind="ExternalInput").ap()
    convb = nc.dram_tensor("convb", [DI, 1], FP32, kind="ExternalInput").ap()
    Wx = nc.dram_tensor("Wx", [DI, 96], BF, kind="ExternalInput").ap()
    Wdt = nc.dram_tensor("Wdt", [DTR, DI], BF, kind="ExternalInput").ap()
    bdt = nc.dram_tensor("bdt", [DI, 1], FP32, kind="ExternalInput").ap()
    Acoef = nc.dram_tensor("Acoef", [DI, DS], FP32, kind="ExternalInput").ap()
    Dsk = nc.dram_tensor("Dsk", [DI, 1], FP32, kind="ExternalInput").ap()
    Wout = nc.dram_tensor("Wout", [DI, C_], BF, kind="ExternalInput").ap()
    outT = nc.dram_tensor("outT", [NB, C_, L], FP32, kind="ExternalOutput").ap()
    # scratch
    z_scr = nc.dram_tensor("z_scr", [NB, ND, 128, L], BF, kind="Internal").ap()
    y_scr = nc.dram_tensor("y_scr", [NB, ND, 128, L], BF, kind="Internal").ap()
    dt_scr = nc.dram_tensor("dt_scr", [NB, ND, 128, L], BF, kind="Internal").ap()
    # B/C per chunk, stored (g, p, t, e)-interleaved for the scan layout
    bc_scr = nc.dram_tensor("bc_scr", [NB, 2, L // TC, DS * TC], BF,
                            kind="Internal").ap()

    io = dict(tokT=tokT, Win=Win, convd=convd, convb=convb, Wx=Wx, Wdt=Wdt,
              bdt=bdt, Acoef=Acoef, Dsk=Dsk, Wout=Wout, outT=outT,
              z_scr=z_scr, y_scr=y_scr, bc_scr=bc_scr, dt_scr=dt_scr)
    with tile.TileContext(nc) as tc:
        with ExitStack() as ctx:
            _emit(ctx, tc, nc, io, scan_op, L=L, TC=TC)
    nc.compile()
    return nc


def _emit(ctx, tc, nc, io, scan_op, *, L, TC):
    tokT, Win, convd, convb, Wx, Wdt, bdt = (
        io["tokT"], io["Win"], io["convd"], io["convb"], io["Wx"], io["Wdt"],
        io["bdt"])
    Acoef, Dsk, Wout, outT = io["Acoef"], io["Dsk"], io["Wout"], io["outT"]
    z_scr, y_scr, bc_scr = io["z_scr"], io["y_scr"], io["bc_scr"]
    dt_scr = io["dt_scr"]

    P = 128
    NCH = L // TC          # t-chunks
    NN = max(1, L // 512)  # matmul n-chunks
    NSZ = L // NN

    # ---- pools ----
    wp = ctx.enter_context(tc.tile_pool(name="weights", bufs=1))
    big = ctx.enter_context(tc.tile_pool(name="big", bufs=4))    # bf16 [128,L]
    af = ctx.enter_context(tc.tile_pool(name="af", bufs=2))      # fp32 scan a
    bh = ctx.enter_context(tc.tile_pool(name="bh", bufs=2))      # bf16 b_/h_
    bcp = ctx.enter_context(tc.tile_pool(name="bcp", bufs=3))    # bf16 B/C rep
    post = ctx.enter_context(tc.tile_pool(name="post", bufs=3))  # hcm/t4
    uvp = ctx.enter_context(tc.tile_pool(name="uvp", bufs=3))    # uv dup
    dtc = ctx.enter_context(tc.tile_pool(name="dtc", bufs=6))   # dt chunks
    xsp = ctx.enter_context(tc.tile_pool(name="xsp", bufs=1))    # xs bf16 resident
    sm = ctx.enter_context(tc.tile_pool(name="small", bufs=2))
    smE = ctx.enter_context(tc.tile_pool(name="smallE", bufs=2))
    pp = ctx.enter_context(tc.tile_pool(name="psum", bufs=2, space="PSUM"))
    pp2 = ctx.enter_context(tc.tile_pool(name="psum2", bufs=2, space="PSUM"))

    # ---- load weights ----
    win0 = wp.tile([P, 2 * DI], BF, tag="win0")
    win1 = wp.tile([C_ - P, 2 * DI], BF, tag="win1")
    nc.sync.dma_start(win0[:], Win[0:P, :])
    nc.sync.dma_start(win1[:], Win[P:C_, :])
    wdt_full = wp.tile([DTR, DI], BF, tag="wdt")
    nc.sync.dma_start(wdt_full[:], Wdt[:])
    wxs, cw3, cb3, bdt3, ac3, dsk3, wo3 = [], [], [], [], [], [], []
    for db in range(ND):
        r = slice(db * P, (db + 1) * P)
        w1 = wp.tile([P, 96], BF, tag=f"wx{db}")
        nc.sync.dma_start(w1[:], Wx[r, :]); wxs.append(w1)
        wconv = []
        for k in range(DC):
            wck = wp.tile([P, P], BF, tag=f"cw{db}_{k}", name=f"cw{db}_{k}")
            nc.sync.dma_start(wck[:], convd[db, k])
            wconv.append(wck)
        cw3.append(wconv)
        w3 = wp.tile([P, 1], FP32, tag=f"cb{db}")
        nc.sync.dma_start(w3[:], convb[r, :]); cb3.append(w3)
        w4 = wp.tile([P, 1], FP32, tag=f"bdt{db}")
        nc.sync.dma_start(w4[:], bdt[r, :]); bdt3.append(w4)
        w5 = wp.tile([P, DS], FP32, tag=f"ac{db}")
        nc.sync.dma_start(w5[:], Acoef[r, :]); ac3.append(w5)
        w6 = wp.tile([P, 1], FP32, tag=f"dsk{db}")
        nc.sync.dma_start(w6[:], Dsk[r, :]); dsk3.append(w6)
        w7 = wp.tile([P, C_], BF, tag=f"wo{db}")
        nc.sync.dma_start(w7[:], Wout[r, :]); wo3.append(w7)

    # persistent scan carries per (db, g): [P, NPAIR, 2]
    hcarry = {}
    for db in range(ND):
        for g in range(NG):
            hcarry[(db, g)] = sm.tile([P, NPAIR, 2], BF, tag=f"carry{db}_{g}",
                                      name=f"carry{db}{g}", bufs=1)

    for j in range(NB):
        # ================= A: in-proj (+ conv interleaved) =================
        tok0 = big.tile([P, L], BF, tag="big")
        tok1 = big.tile([C_ - P, L], BF, tag="big")
        nc.sync.dma_start(tok0[:], tokT[j, 0:P, :])
        nc.sync.dma_start(tok1[:], tokT[j, P:C_, :])

        xs = []
        for m in range(2 * DI // P):   # M-blocks of xz^T; 0..2 -> xi, 3..5 -> z
            if m < ND:
                xi = big.tile([P, L + DC], BF, tag="big")
                nc.scalar.memzero(xi[:, 0:DC])
            mm = slice(m * P, (m + 1) * P)
            for n in range(NN):
                ns = slice(n * NSZ, (n + 1) * NSZ)
                ps = pp.tile([P, NSZ], FP32, tag="ps")
                nc.tensor.matmul(ps[:], win0[:, mm], tok0[:, ns],
                                 start=True, stop=False)
                nc.tensor.matmul(ps[:], win1[:, mm], tok1[:, ns],
                                 start=False, stop=True)
                if m < ND:
                    nc.scalar.copy(xi[:, DC + n * NSZ: DC + (n + 1) * NSZ],
                                   ps[:])
                else:
                    zt = smE.tile([P, NSZ], BF, tag="ztmp", bufs=2)
                    nc.scalar.copy(zt[:], ps[:])
                    sgz = smE.tile([P, NSZ], BF, tag="sgza", bufs=1)
                    nc.scalar.activation(sgz[:], zt[:], AF.Sigmoid)
                    nc.gpsimd.tensor_tensor(zt[:], zt[:], sgz[:], ALU.mult)
                    nc.sync.dma_start(z_scr[j, m - ND, :, ns], zt[:])
            if m < ND:
                # conv on PE via diagonal weight matrices, then silu
                db = m
                x_ = xsp.tile([P, L], BF, tag=f"xs{db}")
                for n in range(NN):
                    ns = slice(n * NSZ, (n + 1) * NSZ)
                    psc = pp.tile([P, NSZ], FP32, tag="psc")
                    for k in range(DC):
                        nc.tensor.matmul(
                            psc[:], cw3[db][k][:],
                            xi[:, 1 + k + n * NSZ: 1 + k + n * NSZ + NSZ],
                            start=(k == 0), stop=(k == DC - 1))
                    sgt = smE.tile([P, NSZ], BF, tag="sgt", bufs=1)
                    nc.scalar.activation(sgt[:], psc[:], AF.Sigmoid,
                                         bias=cb3[db])
                    nc.vector.scalar_tensor_tensor(x_[:, ns], psc[:],
                                                   cb3[db][:], sgt[:],
                                                   ALU.add, ALU.mult)
                xs.append(x_)

        # ================= C: dbc, dt =================
        for n in range(NN):
            ns = slice(n * NSZ, (n + 1) * NSZ)
            psd = pp2.tile([96, NSZ], FP32, tag="psd")
            for db in range(ND):
                nc.tensor.matmul(psd[:], wxs[db][:], xs[db][:, ns],
                                 start=(db == 0), stop=(db == ND - 1))
            dtl = sm.tile([DTR, NSZ], BF, tag="dtl", bufs=2)
            nc.scalar.copy(dtl[:], psd[0:DTR, :])
            bt = smE.tile([DS, NSZ], BF, tag="bt", bufs=1)
            ct = smE.tile([DS, NSZ], BF, tag="ct", bufs=1)
            nc.vector.tensor_copy(bt[:], psd[32:32 + DS, :])
            nc.vector.tensor_copy(ct[:], psd[64:64 + DS, :])
            nc.sync.dma_start(
                bc_scr[j, 0, n].rearrange("(s t) -> s t", s=DS), bt[:])
            nc.sync.dma_start(
                bc_scr[j, 1, n].rearrange("(s t) -> s t", s=DS), ct[:])
            for db in range(ND):
                psm = pp.tile([P, NSZ], FP32, tag="ps")
                nc.tensor.matmul(psm[:], wdt_full[:, db * P:(db + 1) * P],
                                 dtl[:], start=True, stop=True)
                ec = smE.tile([P, NSZ], FP32, tag="esp", bufs=1)
                nc.scalar.activation(ec[:], psm[:], AF.Exp, bias=bdt3[db])
                dt_c = smE.tile([P, NSZ], BF, tag="dtc_w", bufs=2)
                nc.scalar.activation(dt_c[:], ec[:], AF.Ln, bias=1.0)
                nc.sync.dma_start(dt_scr[j, db, :, ns], dt_c[:])

        # ================= D: selective scan =================
        for ch in range(NCH):
            cs = slice(ch * TC, (ch + 1) * TC)
            # dt chunks back from DRAM; u = dt*xs duplicated into (t, e) pairs
            uvd, dtch = [], []
            for db in range(ND):
                dt_ = dtc.tile([P, TC], BF, tag="dtc_r")
                nc.sync.dma_start(dt_[:], dt_scr[j, db, :, cs])
                dtch.append(dt_)
                uv = uvp.tile([P, TC, 2], BF, tag="uv")
                nc.vector.tensor_tensor(
                    uv[:],
                    dt_[:].unsqueeze(2).broadcast_to((P, TC, 2)),
                    xs[db][:, cs].unsqueeze(2).broadcast_to((P, TC, 2)),
                    ALU.mult)
                uvd.append(uv)
            ysums = {}
            for g in range(NG):
                bcont = bcp.tile([P, 8, TC], BF, tag="bccont", bufs=2)
                ccont = bcp.tile([P, 8, TC], BF, tag="bccont", bufs=2)
                gsl = slice(8 * g * TC, 8 * (g + 1) * TC)
                nc.sync.dma_start(
                    bcont[:], bc_scr[j, 0, ch][gsl]
                    .rearrange("(s t) -> s t", s=8)
                    .unsqueeze(0).broadcast_to((P, 8, TC)))
                nc.sync.dma_start(
                    ccont[:], bc_scr[j, 1, ch][gsl]
                    .rearrange("(s t) -> s t", s=8)
                    .unsqueeze(0).broadcast_to((P, 8, TC)))
                # interleave (s, t) -> (p, t, e) on-chip
                brep = bcp.tile([P, NPAIR, 2 * TC], BF, tag="bcitl", bufs=3)
                crep = bcp.tile([P, NPAIR, 2 * TC], BF, tag="bcitl", bufs=3)
                nc.vector.tensor_copy(
                    brep[:].rearrange("x p (t e) -> x p t e", e=2),
                    bcont[:].rearrange("x (p e) t -> x p t e", e=2))
                nc.scalar.copy(
                    crep[:].rearrange("x p (t e) -> x p t e", e=2),
                    ccont[:].rearrange("x (p e) t -> x p t e", e=2))
                for db in range(ND):
                    # a coefficients: exp(A[s] * dt), strided (t, e) writes
                    a_ = af.tile([P, NPAIR, PLEN], FP32, tag="a")
                    nc.scalar.memzero(a_[:, :, 0:2])
                    av = a_[:, :, 2:].rearrange("p q (t e) -> p q t e", e=2)
                    din = dtch[db][:].unsqueeze(2)
                    for p_ in range(NPAIR):
                        for e in range(2):
                            s = 8 * g + 2 * p_ + e
                            nc.scalar.activation(av[:, p_, :, e:e + 1], din,
                                                 AF.Exp,
                                                 scale=ac3[db][:, s:s + 1])
                    # b = u*B (+ carry boundary)
                    b_ = bh.tile([P, NPAIR, PLEN], BF, tag="bh")
                    nc.vector.tensor_tensor(
                        b_[:, :, 2:],
                        uvd[db][:].rearrange("p t e -> p (t e)").unsqueeze(1)
                        .broadcast_to((P, NPAIR, 2 * TC)),
                        brep[:], ALU.mult)
                    if ch == 0:
                        nc.vector.memset(b_[:, :, 0:2], 0.0)
                    else:
                        nc.vector.tensor_copy(b_[:, :, 0:2],
                                              hcarry[(db, g)][:])
                    # dual interleaved scan
                    h_ = bh.tile([P, NPAIR, PLEN], BF, tag="bh")
                    nc.vector._custom_dve(
                        scan_op,
                        out=h_[:].rearrange("p q t -> p (q t)"),
                        in0=a_[:].rearrange("p q t -> p (q t)"),
                        in1=b_[:].rearrange("p q t -> p (q t)"),
                        s0=0.0, s1=0.0)
                    nc.vector.tensor_copy(hcarry[(db, g)][:],
                                          h_[:, :, 2 * TC:2 * TC + 2])
                    # y partials: sum_s h*C
                    hcm = post.tile([P, NPAIR, 2 * TC], BF, tag="hcm", bufs=1)
                    nc.vector.tensor_tensor(hcm[:], h_[:, :, 2:], crep[:],
                                            ALU.mult)
                    hv = hcm[:].rearrange("p q (t e) -> p q t e", e=2)
                    t4 = post.tile([P, NPAIR, TC], BF, tag="t4", bufs=1)
                    nc.vector.tensor_tensor(t4[:], hv[:, :, :, 0],
                                            hv[:, :, :, 1], ALU.add)
                    t2 = sm.tile([P, 2, TC], BF, tag="t2", bufs=1)
                    nc.vector.tensor_tensor(t2[:], t4[:, 0:2, :],
                                            t4[:, 2:4, :], ALU.add)
                    yg = sm.tile([P, TC], BF, tag=f"yg{db}_{g}", bufs=1)
                    nc.vector.tensor_tensor(yg[:], t2[:, 0, :], t2[:, 1, :],
                                            ALU.add)
                    ysums[(db, g)] = yg
            for db in range(ND):
                ysum = sm.tile([P, TC], BF, tag="ysum", bufs=1)
                nc.vector.tensor_tensor(ysum[:], ysums[(db, 0)][:],
                                        ysums[(db, 1)][:], ALU.add)
                # y = ys + xs*D -> bf16 -> DRAM
                ybf = sm.tile([P, TC], BF, tag="ybf", bufs=2)
                nc.vector.scalar_tensor_tensor(ybf[:], xs[db][:, cs],
                                               dsk3[db][:], ysum[:],
                                               ALU.mult, ALU.add)
                nc.sync.dma_start(y_scr[j, db, :, cs], ybf[:])

    # ================= E: gate + out-proj =================
    for j in range(NB):
        for n in range(NN):
            ns = slice(n * NSZ, (n + 1) * NSZ)
            ygs = []
            for db in range(ND):
                zt = smE.tile([P, NSZ], BF, tag="ze", bufs=2)
                nc.sync.dma_start(zt[:], z_scr[j, db, :, ns])
                yt = smE.tile([P, NSZ], BF, tag="ye", bufs=2)
                nc.sync.dma_start(yt[:], y_scr[j, db, :, ns])
                nc.gpsimd.tensor_tensor(yt[:], yt[:], zt[:], ALU.mult)
                ygs.append(yt)
            for m in range(2):
                msz = P if m == 0 else C_ - P
                mm = slice(m * P, m * P + msz)
                pso = pp2.tile([msz, NSZ], FP32, tag="pso")
                for db in range(ND):
                    nc.tensor.matmul(pso[:], wo3[db][:, mm], ygs[db][:],
                                     start=(db == 0), stop=(db == ND - 1))
                ot = smE.tile([msz, NSZ], FP32, tag="oe", bufs=1)
                nc.scalar.copy(ot[:], pso[:])
                nc.sync.dma_start(outT[j, mm, ns], ot[:])


# ---------------- host side ----------------

_CACHE = {}
PROFILE = False
PROFILE_KW = {}


def _get_nc():
    if "nc" not in _CACHE:
        _CACHE["nc"] = build_nc()
    return _CACHE["nc"]


def _permute_toks(x, idx):
    """x: [C, H, W] fp32 -> 4 direction token maps, each [C, L] (c-major)."""
    c = x.shape[0]
    row = x.reshape(c, -1)
    col = x.transpose(0, 2, 1).reshape(c, -1)
    diag = row[:, idx]
    anti = x[:, :, ::-1].reshape(c, -1)[:, idx]
    return [row, col, diag, anti]


def _unpermute(outs, inv_idx, h, w):
    """outs: list of 4 [C, L] -> sum of un-permuted direction outputs."""
    c = outs[0].shape[0]
    row_f = outs[0].reshape(c, h, w)
    col_f = outs[1].reshape(c, w, h).transpose(0, 2, 1)
    diag_f = outs[2][:, inv_idx].reshape(c, h, w)
    anti_f = outs[3][:, inv_idx].reshape(c, h, w)[:, :, ::-1]
    return row_f + col_f + diag_f + anti_f


def _pack_convd(cw):
    """Per d-block, per tap: diag(conv_w[:, k]) as bf16 PE weights."""
    out = np.zeros((ND, DC, 128, 128), np.float32)
    for db in range(ND):
        for k in range(DC):
            np.fill_diagonal(out[db, k], cw[db * 128:(db + 1) * 128, k])
    return out.astype(BF16)


def _pack_wx(wx):
    """Pad W_x columns so dt/B/C rows land at PSUM partitions 0/32/64."""
    out = np.zeros((DI, 96), np.float32)
    out[:, 0:DTR] = wx[:, 0:DTR]
    out[:, 32:32 + DS] = wx[:, DTR:DTR + DS]
    out[:, 64:64 + DS] = wx[:, DTR + DS:]
    return out.astype(BF16)


def kernel(x, W_in, conv_w, conv_b, W_x, W_dt, b_dt, A_log, D_skip, W_out,
           idx, inv_idx):
    x = np.asarray(x, np.float32)
    idx = np.asarray(idx, np.int32)
    inv_idx = np.asarray(inv_idx, np.int32)
    A = -np.exp(np.asarray(A_log, np.float32))        # [4, DI, DS]
    conv_b = np.asarray(conv_b, np.float32)
    b_dt = np.asarray(b_dt, np.float32)
    D_skip = np.asarray(D_skip, np.float32)

    nc = _get_nc()
    in_maps = []
    for core in range(N_CORES):
        d = core // 2      # direction
        bh = core % 2      # batch half
        toks = np.empty((NB, C_, L), BF16)
        for jb in range(NB):
            b = bh * NB + jb
            toks[jb] = _permute_toks(x[b], idx)[d].astype(BF16)
        in_maps.append(dict(
            tokT=toks,
            Win=np.asarray(W_in[d], np.float32).astype(BF16),
            convd=_pack_convd(np.asarray(conv_w[d], np.float32)),
            convb=np.ascontiguousarray(conv_b[d].reshape(DI, 1)),
            Wx=_pack_wx(np.asarray(W_x[d], np.float32)),
            Wdt=np.asarray(W_dt[d], np.float32).astype(BF16),
            bdt=np.ascontiguousarray(b_dt[d].reshape(DI, 1)),
            Acoef=np.ascontiguousarray(A[d]),
            Dsk=np.ascontiguousarray(D_skip[d].reshape(DI, 1)),
            Wout=np.asarray(W_out[d], np.float32).astype(BF16),
        ))

    res = run_bass_kernel_spmd(nc, in_maps, list(range(N_CORES)),
                               trace=PROFILE, **PROFILE_KW)
    _CACHE["last_exec_ns"] = res.exec_time_ns
    outs = res.results

    # gather: per batch b, the 4 direction outputs live on cores d*2 + b//2
    acc = np.zeros((B_, C_, H_, W_), np.float32)
    for b in range(B_):
        bh, jb = b // NB, b % NB
        douts = [np.asarray(outs[d * 2 + bh]["outT"][jb], np.float32)
                 for d in range(4)]
        acc[b] = _unpermute(douts, inv_idx, H_, W_)
    gate = 1.0 / (1.0 + np.exp(-0.25 * acc))
    return x * gate
